# revision 1
# baseline (speedup 1.0000x reference)
"""Trainium2 Bass kernel for nn_InvariantHeadviaTP.

Reference computation (after dead-code elimination -- y1/y2/gates are never
used by the output):
    x0   = node_vec[:, :128]                  # [N, 128]
    a    = node_embedding                     # [N, 16]
    s0   = einsum('ni,na,iak->nk', x0, a, W1_l0[:, :, :128]) / sqrt(2048) + b1[:128]
    scal = silu(s0)                           # [N, 128]
    mid  = einsum('ni,na,iak->nk', scal, a, W2) / sqrt(2048) + b2   # [N, 16]
    h    = silu(mid @ W3 / 4 + b3)            # [N, 16]
    out  = h @ W4 / 4 + b4                    # [N, 1]

Strategy (data-parallel, 2048 nodes/core, transposed layout: features on
partitions, nodes on the free dim):

s0 path -- mixed (i,a) chunking. The contraction index c=(i,a) [128*16=2048]
is split into 16 chunks of 128 = (32 i's) x (4 a's); chunk (bi,bj) has
partition p = 4*i_loc + a_loc. The elementwise operand
U_c[p,n] = x0[32bi+p//4, n] * a[4bj+p%4, n] is built on DVE from
  x0rep[h][bi][p,n]  = x0t[32bi+p//4, n]   (x0 rows repeated 4x, stride-0 DMA)
  pats[h][p,(bj,n)]  = aT[4bj+p%4, n]      (a rows tiled 32x,   stride-0 DMA)
so the replicated-broadcast traffic is 4 MB/core instead of the naive 8 MB
(a replicated to all 128 partitions). One DVE op per (bi, half) builds 4
chunks at once ([128, 4, 1024], hits the 2x bf16 DVE mode); s0 accumulates
over the 16 chunks in PSUM via 16 matmuls per 512-node tile. All input
tiles are per-half so dependency tracking releases compute as soon as that
half's DMAs land; DMAs are issued critical-path-first across the three DGE
queues (sync / scalar / gpsimd).

silu is a single hardware Activation op (Silu shares the HW table with
Identity, so no table reloads). CoreSim has no Silu LUT, so sim_silu=True
builds an Identity+Sigmoid+mul equivalent for simulation.

mid path -- only 16 outputs, so instead of another 16-matmul bilinear:
  g[(a,k'),n] = sum_i scal[i,n] W2[i,a,k']        (matmuls, stat [128,128])
  v = g * patm,  patm[(a,k'),n] = aT[a,n]         (elementwise)
  h_pre[k2,n] = sum_(a,k') v * W3[k',k2]          (matmuls, W3 fused into
                                                   the a-sum selector)
with b2 folded into b3' = b3 + b2 @ W3s host-side, b4 applied as the bias of
the final PSUM->SBUF copy. For all but the last half the g*patm multiply
goes scalar-copy(g)->bf16-DVE-mul to keep the (critical) DVE backbone short;
the last half multiplies f32 PSUM directly on DVE to keep its tail chain
short.

PSUM (8 banks): per-tile s0 accumulators share bank slots with the
half-wide g tiles (disjoint lifetimes, 2x2 banks), h_pre [16,1024] 2 banks,
out [1,1024] 2 banks.
"""

import numpy as np
import ml_dtypes
from contextlib import ExitStack

import concourse.bass as bass
import concourse.mybir as mybir
import concourse.tile as tile
from concourse import bacc
from concourse.bass import ts
from concourse.bass_utils import run_bass_kernel_spmd

N_CORES = 8
N_FULL = 16384
NSH = N_FULL // N_CORES          # 2048 nodes per core
A = 16                           # attr dim
M0 = 128                         # MUL0 (scalar channels)
FREE = 512                       # node tile (free dim) per PSUM tile
HALF = 1024                      # nodes per half-phase
SCALE = 1.0 / np.sqrt(M0 * A)    # path normalization of both fctp einsums
BF16 = ml_dtypes.bfloat16

AF = mybir.ActivationFunctionType
F32 = mybir.dt.float32
DBF16 = mybir.dt.bfloat16


def build_nc(nsh: int = NSH, num_devices: int = N_CORES, sim_silu: bool = False):
    assert nsh % HALF == 0
    n_halves = nsh // HALF

    nc = bacc.Bacc(
        "TRN2",
        target_bir_lowering=False,
        debug=False,
        enable_asserts=False,
        num_devices=num_devices,
    )

    x0t = nc.dram_tensor("x0t", [M0, nsh], DBF16, kind="ExternalInput").ap()
    at = nc.dram_tensor("at", [A, nsh], DBF16, kind="ExternalInput").ap()
    w0f = nc.dram_tensor("w0f", [M0, 16 * M0], DBF16, kind="ExternalInput").ap()
    w2g = nc.dram_tensor("w2g", [M0, 2 * M0], DBF16, kind="ExternalInput").ap()
    s3 = nc.dram_tensor("s3", [2 * M0, A], DBF16, kind="ExternalInput").ap()
    w4a = nc.dram_tensor("w4a", [A, 1], DBF16, kind="ExternalInput").ap()
    b1 = nc.dram_tensor("b1", [M0, 1], F32, kind="ExternalInput").ap()
    b3p = nc.dram_tensor("b3p", [A, 1], F32, kind="ExternalInput").ap()
    b4 = nc.dram_tensor("b4", [1, 1], F32, kind="ExternalInput").ap()
    outt = nc.dram_tensor("outt", [1, nsh], F32, kind="ExternalOutput").ap()

    with tile.TileContext(nc) as tc, ExitStack() as ctx:
        consts = ctx.enter_context(tc.tile_pool(name="consts", bufs=1))

        # -- SBUF residents (per-half tiles for exact DMA dep granularity) --
        x0rep = [[None] * 4 for _ in range(n_halves)]
        pats = [None] * n_halves      # [128, 4(bj), HALF]
        for h in range(n_halves):
            for bi in range(4):
                x0rep[h][bi] = consts.tile([M0, HALF], DBF16,
                                           name=f"x0rep{h}_{bi}")
            pats[h] = consts.tile([M0, 4, HALF], DBF16, name=f"pats{h}")
        p16 = consts.tile([M0, nsh], DBF16)   # p16[p, n] = aT[p % 16, n]
        w0_sb = consts.tile([M0, 16 * M0], DBF16)
        w2g_sb = consts.tile([M0, 2 * M0], DBF16)
        s3_sb = consts.tile([M0, 2 * A], DBF16)
        w4a_sb = consts.tile([A, 1], DBF16)
        b1_sb = consts.tile([M0, 1], F32)
        b3p_sb = consts.tile([A, 1], F32)
        b4_sb = consts.tile([1, 1], F32)
        scal_sb = consts.tile([M0, nsh], DBF16)
        hb_all = consts.tile([A, nsh], DBF16)
        ob = consts.tile([1, nsh], F32)
        gc_sb = consts.tile([M0, 2 * HALF], DBF16)   # bf16 copies of g (B1)

        # -- prefetch DMAs: critical-path first, spread over 3 queues -------
        # x0rep[h][bi][p, n] = x0t[32*bi + p//4, n+h*HALF]
        # pats[h][p, bj, n]  = aT[4*bj + p%4, n+h*HALF]
        def load_x0rep(h, bi, eng):
            eng.dma_start(
                x0rep[h][bi][:],
                x0t[32 * bi:32 * bi + 32, ts(h, HALF)]
                .unsqueeze(1).broadcast_to([32, 4, HALF]),
            )

        def load_pats(h, bj, eng):
            eng.dma_start(
                pats[h][:, bj, :],
                at[4 * bj:4 * bj + 4, ts(h, HALF)]
                .unsqueeze(0).broadcast_to([32, 4, HALF]),
            )

        # Priority order: A1's inputs, then A2's, then phase-B patterns.
        # sync carries x0rep, scalar/gpsimd split pats so the two A1 halves
        # of the first U op unblock as early as possible.
        load_x0rep(0, 0, nc.sync)
        nc.gpsimd.dma_start(w0_sb[:, 0:4 * M0], w0f[:, 0:4 * M0])
        load_pats(0, 0, nc.scalar)
        load_pats(0, 1, nc.scalar)
        load_pats(0, 2, nc.gpsimd)
        load_pats(0, 3, nc.gpsimd)
        load_x0rep(0, 1, nc.sync)
        load_x0rep(0, 2, nc.sync)
        load_x0rep(0, 3, nc.sync)
        nc.gpsimd.dma_start(w0_sb[:, 4 * M0:8 * M0], w0f[:, 4 * M0:8 * M0])
        nc.gpsimd.dma_start(w0_sb[:, 8 * M0:12 * M0], w0f[:, 8 * M0:12 * M0])
        nc.gpsimd.dma_start(w0_sb[:, 12 * M0:16 * M0], w0f[:, 12 * M0:16 * M0])
        load_pats(1, 0, nc.scalar)
        load_pats(1, 1, nc.scalar)
        load_x0rep(1, 0, nc.sync)
        load_x0rep(1, 1, nc.sync)
        load_x0rep(1, 2, nc.sync)
        load_x0rep(1, 3, nc.sync)
        load_pats(1, 2, nc.gpsimd)
        load_pats(1, 3, nc.scalar)
        nc.gpsimd.dma_start(w2g_sb[:], w2g)
        nc.scalar.dma_start(b1_sb[:], b1)
        nc.gpsimd.dma_start(s3_sb[:, 0:A], s3[0:M0, :])
        nc.gpsimd.dma_start(s3_sb[:, A:2 * A], s3[M0:2 * M0, :])
        nc.gpsimd.dma_start(w4a_sb[:], w4a)
        nc.scalar.dma_start(b3p_sb[:], b3p)
        nc.scalar.dma_start(b4_sb[:], b4)
        # p16[p, n] = aT[p % 16, n] (phase B; g rows are k'-major, a-minor)
        for hh in range(n_halves):
            nc.gpsimd.dma_start(
                p16[:, ts(hh, HALF)],
                at[:, ts(hh, HALF)].unsqueeze(0).broadcast_to([8, A, HALF]),
            )

        u_pool = ctx.enter_context(tc.tile_pool(name="u", bufs=5))
        v_pool = ctx.enter_context(tc.tile_pool(name="v", bufs=2))
        sim_pool = (
            ctx.enter_context(tc.tile_pool(name="simtmp", bufs=2))
            if sim_silu else None
        )
        # PSUM budget (8 banks): s0 accumulators 2x1 + g (one 4-bank tile,
        # whose slot the out stage reuses) + h_pre 2.
        ps_s0 = ctx.enter_context(tc.tile_pool(name="ps_s0", bufs=1, space="PSUM"))
        ps_gg = ctx.enter_context(tc.tile_pool(name="ps_gg", bufs=1, space="PSUM"))
        ps_h = ctx.enter_context(tc.tile_pool(name="ps_h", bufs=1, space="PSUM"))

        def silu(out_ap, in_ap, bias_ap, np_, nf):
            if not sim_silu:
                nc.scalar.activation(out_ap, in_ap, AF.Silu, bias=bias_ap)
            else:
                pre = sim_pool.tile([M0, HALF], DBF16, tag="pre", name="pre")
                sig = sim_pool.tile([M0, HALF], DBF16, tag="sig", name="sig")
                nc.scalar.activation(pre[0:np_, 0:nf], in_ap, AF.Identity,
                                     bias=bias_ap)
                nc.scalar.activation(sig[0:np_, 0:nf], in_ap, AF.Sigmoid,
                                     bias=bias_ap)
                nc.vector.tensor_mul(out_ap, pre[0:np_, 0:nf], sig[0:np_, 0:nf])

        # Three pipeline phases: (input-half, col-offset, width). Phases 1-2
        # split half 1 so the tail chain runs on 512-wide ops and the final
        # s0-stop comes earlier; no DMA changes (they slice half-1's tiles).
        phases = [(0, 0, HALF), (1, 0, FREE), (1, FREE, FREE)]
        for pi, (ht, off, W) in enumerate(phases):
            last = pi == len(phases) - 1
            base = ht * HALF + off        # global node offset
            n_t2 = W // FREE

            # ---- phase A: build U (DVE), accumulate s0 (PE) ----
            s0_ps = [
                ps_s0.tile([M0, FREE], F32, tag=f"s0{(base // FREE + t2) % 2}",
                           name=f"s0_{pi}_{t2}")
                for t2 in range(n_t2)
            ]
            u_tiles = []
            for bi in range(4):
                u = u_pool.tile([M0, 4, W], DBF16, tag="u", name=f"u{pi}_{bi}")
                x0s = x0rep[ht][bi][:, off:off + W]
                if (pi == 0 and bi == 0) or (last and bi == 3):
                    # per-bj sub-ops: at the start they unblock on the first
                    # pattern DMA; at the end they let chunk c=12+bj's
                    # matmuls start early, pulling the final s0-stop in
                    for bj in range(4):
                        nc.vector.tensor_mul(
                            u[:, bj:bj + 1, :],
                            x0s.unsqueeze(1).broadcast_to([M0, 1, W]),
                            pats[ht][:, bj:bj + 1, off:off + W],
                        )
                else:
                    nc.vector.tensor_mul(
                        u[:],
                        x0s.unsqueeze(1).broadcast_to([M0, 4, W]),
                        pats[ht][:, :, off:off + W],
                    )
                u_tiles.append(u)

            for c in range(16):
                bi, bj = c >> 2, c & 3
                for t2 in range(n_t2):
                    nc.tensor.matmul(
                        s0_ps[t2][:],
                        w0_sb[:, ts(c, M0)],
                        u_tiles[bi][:, bj, ts(t2, FREE)],
                        start=(c == 0),
                        stop=(c == 15),
                    )

            # ---- phase B: mid path for this phase's nodes ----
            for t2 in range(n_t2):
                silu(scal_sb[:, base + t2 * FREE:base + (t2 + 1) * FREE],
                     s0_ps[t2][:], b1_sb[:], M0, FREE)

            g = ps_gg.tile([M0, 2, W], F32, tag="gg", name=f"g{pi}")
            for gi in range(2):
                for t2 in range(n_t2):
                    nc.tensor.matmul(
                        g[:, gi, ts(t2, FREE)],
                        w2g_sb[:, ts(gi, M0)],
                        scal_sb[:, base + t2 * FREE:base + (t2 + 1) * FREE],
                        start=True, stop=True,
                    )

            v = v_pool.tile([M0, 2, W], DBF16, tag="v", name=f"v{pi}")
            pm = p16[:, base:base + W].unsqueeze(1).broadcast_to([M0, 2, W])
            if last:
                # tail: one DVE op straight off f32 PSUM, shortest chain
                nc.vector.tensor_mul(v[:], g[:], pm)
            else:
                # keep the DVE backbone for the U-build: scalar converts g
                # to bf16, DVE multiplies at the 2x bf16 rate
                gc3 = gc_sb[:, 0:2 * W].rearrange("p (g n) -> p g n", g=2)
                nc.scalar.activation(gc3, g[:], AF.Identity)
                nc.vector.tensor_mul(v[:], gc3, pm)

            h_ps = ps_h.tile([A, W], F32, tag="h", name=f"h{pi}")
            for t2 in range(n_t2):
                for gi in range(2):
                    nc.tensor.matmul(
                        h_ps[:, ts(t2, FREE)], s3_sb[:, ts(gi, A)],
                        v[:, gi, ts(t2, FREE)],
                        start=(gi == 0), stop=(gi == 1),
                    )
            silu(hb_all[:, base:base + W], h_ps[:], b3p_sb[:], A, W)

            # o lives in the h tag so the next phase's g (gg tag) only
            # waits on this phase's v-read, not on the out-act
            o_ps = ps_h.tile([1, W], F32, tag="h", name=f"o{pi}")
            for t2 in range(n_t2):
                nc.tensor.matmul(
                    o_ps[:, ts(t2, FREE)], w4a_sb[:],
                    hb_all[:, base + t2 * FREE:base + (t2 + 1) * FREE],
                    start=True, stop=True,
                )
            nc.scalar.activation(ob[:, base:base + W], o_ps[:], AF.Identity,
                                 bias=b4_sb[:])
            nc.sync.dma_start(outt[:, base:base + W], ob[:, base:base + W])

    nc.compile()
    return nc


def prep_host(inputs: dict, nsh: int = NSH, n_cores: int = N_CORES):
    """Host-side prep: slice/transpose/cast inputs, build per-core in_maps."""
    node_vec = np.asarray(inputs["node_vec"], dtype=np.float32)
    node_embedding = np.asarray(inputs["node_embedding"], dtype=np.float32)
    W1_l0 = np.asarray(inputs["W1_l0"], dtype=np.float32)
    b1 = np.asarray(inputs["b1"], dtype=np.float32)
    W2 = np.asarray(inputs["W2"], dtype=np.float32)
    b2 = np.asarray(inputs["b2"], dtype=np.float32)
    W3 = np.asarray(inputs["W3"], dtype=np.float32)
    b3 = np.asarray(inputs["b3"], dtype=np.float32)
    W4 = np.asarray(inputs["W4"], dtype=np.float32)
    b4 = np.asarray(inputs["b4"], dtype=np.float32)

    x0T = np.ascontiguousarray(node_vec[:, :M0].T).astype(BF16)      # [128, N]
    aT = np.ascontiguousarray(node_embedding.T).astype(BF16)         # [16, N]

    W0s = W1_l0[:, :, :M0] * SCALE                                   # [128,16,128]
    # chunk (bi,bj): stationary rows p=(i_loc*4 + a_loc), cols (c*128 + k)
    w0r = W0s.reshape(4, 32, 4, 4, M0)            # [bi, i_loc, bj, a_loc, k]
    w0f = np.ascontiguousarray(
        w0r.transpose(1, 3, 0, 2, 4).reshape(M0, 16 * M0)
    ).astype(BF16)

    W3s = W3 / np.sqrt(A)                                            # [16, 16]
    # g rows are (k'-major, a-minor): col j = k'_loc*16 + a
    w2g = np.ascontiguousarray(
        (W2 * SCALE).transpose(0, 2, 1).reshape(M0, A * A)
    ).astype(BF16)
    # sel stationary rows (k', a): S3[p, k2] = W3s[p//16, k2], 2 tiles
    s3h = np.ascontiguousarray(np.repeat(W3s, A, axis=0)).astype(BF16)  # [256, 16]
    w4a = (W4 / np.sqrt(A)).astype(BF16)                             # [16, 1]
    b3ph = (b3 + b2 @ W3s).reshape(A, 1).astype(np.float32)

    shared = {
        "w0f": w0f, "w2g": w2g, "s3": s3h, "w4a": w4a,
        "b1": np.ascontiguousarray(b1[:M0].reshape(M0, 1)),
        "b3p": b3ph,
        "b4": np.ascontiguousarray(b4.reshape(1, 1)),
    }
    in_maps = []
    for c in range(n_cores):
        sl = slice(c * nsh, (c + 1) * nsh)
        in_maps.append({
            "x0t": np.ascontiguousarray(x0T[:, sl]),
            "at": np.ascontiguousarray(aT[:, sl]),
            **shared,
        })
    return in_maps


_NC_CACHE = {}


def _get_nc():
    if "nc" not in _NC_CACHE:
        _NC_CACHE["nc"] = build_nc()
    return _NC_CACHE["nc"]


def kernel_with_results(trace: bool = False, **inputs):
    nc = _get_nc()
    in_maps = prep_host(inputs)
    res = run_bass_kernel_spmd(
        nc, in_maps, core_ids=list(range(N_CORES)), trace=trace,
    )
    out = np.empty((N_FULL, 1), dtype=np.float32)
    for c in range(N_CORES):
        out[c * NSH:(c + 1) * NSH, 0] = res.results[c]["outt"][0]
    return out, res


def kernel(**inputs) -> np.ndarray:
    out, _ = kernel_with_results(trace=False, **inputs)
    return out

